# revision 5
# baseline (speedup 1.0000x reference)
# kernel.py — Trainium2 Bass kernel for nn_MultiHeadCrossAttention
#
# Sharding: pure data-parallel over batch. 8 cores x 2 batches each; zero
# collectives. Each core computes: two projections (+LN+l2norm), 16-head
# bidirectional cross-attention, residual, concat, final per-head LN.
#
# Core dataflow (per NeuronCore, B_LOC=2 batches, NTOK=1024 tokens):
#   1. Host uploads x pre-transposed (feature-major) in fp16, so the PE
#      does no input transposes. Projection matmuls in fp16 (fp32/fp32r
#      hit a walrus 1-wait LDW codegen limit), fp32 PSUM accum.
#   2. LN stats via bn_stats/bn_aggr on DVE. For the default affine
#      (g=1,b=0), LN + l2norm + 0.125 scaling collapse algebraically to
#      (x-mu) * 1/(256*sqrt(var)) — eps cancels exactly. The normalize
#      itself runs on ScalarE as Copy(x*sv + (-mu*sv)) with per-partition
#      scale/bias; sv = exp(-0.5*ln(65536*var)) on ACT (Ln/Exp share a
#      table set — no ACT table thrashing with the softmax exps).
#   3. ip/tp stored token-major fp16; hid-major ipT/tpT produced by
#      SBUF->SBUF DMA-transposes per (hid-block, token-tile) chunk so the
#      transposes stream behind the projections (no DRAM bounce, no
#      all-tiles barrier).
#   4. Attention per (batch, head-pair, direction) with the Gram trick:
#      scores^T = (G^T ctT)^T ciT where G = 0.125*Wk^T Wq is folded on
#      host; u = G^T @ ctT (K=64 matmuls packed 2-heads via partition
#      offsets), scoresT = u^T @ ciT, exp on ACT (scores ~1e-5: no
#      max-sub needed), p^T fp16. v and out-proj fold into one matmul via
#      host-computed Wvo = (out_w @ Wv)^T; out-matmuls use p^T blocks as
#      stationary against [vo | ones] so the softmax denominator l rides
#      along as column 64.
#   5. comb = ci + attn/l via scalar_tensor_tensor (recip(l) per-partition)
#      on DVE, with accum_out riding for the final-LN sums.
#   6. Final per-head LN over 2D=128 (eps=1e-5 load-bearing) done in 16
#      chunks of (sqt, 8 heads); batch 0's chunks are interleaved into
#      batch 1's attention loop so the tail is only batch 1's chunks.
#      Square + the two normalize passes run on GpSimd (otherwise idle),
#      reduces on DVE, rstd via Ln/Exp on ACT. Output written token-major
#      [B_LOC, S, H, 2D] (contiguous 4KB descriptors) and transposed back
#      to [B, H, S, 2D] on host.

import numpy as np
import ml_dtypes

import concourse.bass as bass
import concourse.mybir as mybir
import concourse.tile as tile
from concourse import bacc
from concourse.bass_utils import run_bass_kernel_spmd

AF = mybir.ActivationFunctionType
ALU = mybir.AluOpType
F32 = mybir.dt.float32
F16 = mybir.dt.float16

NCORES = 8
H = 16
D = 64
S = 512
B = 16
B_LOC = B // NCORES          # 2 batches per core
NTOK = B_LOC * S             # 1024 tokens per core
HID = H * D                  # 1024
HP = H // 2                  # 8 head pairs

USE_GPSIMD = True            # final-LN square+normalize on GpSimd


def _build_program(di_k: int, dt_k: int):
    """Build the single-core Bass/Tile program. di_k/dt_k = number of
    128-wide feature tiles for the image/text projections (6/4 normally,
    7/5 when a projection bias is folded in as an extra padded block)."""
    nc = bacc.Bacc()
    DI = di_k * 128
    DT = dt_k * 128

    xT_img = nc.declare_dram_parameter("xT_img", [DI, NTOK], F16, isOutput=False)
    xT_txt = nc.declare_dram_parameter("xT_txt", [DT, NTOK], F16, isOutput=False)
    w_imgT = nc.declare_dram_parameter("w_imgT", [DI, HID], F16, isOutput=False)
    w_txtT = nc.declare_dram_parameter("w_txtT", [DT, HID], F16, isOutput=False)
    # G matrices: [ (h%2)*64+d_kv , hp, dir, d_q ] ; 0.125 * Wk^T @ Wq
    qk_g = nc.declare_dram_parameter("qk_g", [128, HP, 2, D], F16, isOutput=False)
    # vo weights: [ (h%2)*64+d_kv , hp, dir, f ] ; (out_w @ Wv)^T
    vo_w = nc.declare_dram_parameter("vo_w", [128, HP, 2, D], F16, isOutput=False)
    # token-major output; host transposes back to [B, H, S, 2D]
    out = nc.declare_dram_parameter("out", [B_LOC, S, H, 2 * D], F32, isOutput=True)

    with tile.TileContext(nc) as tc:
        import contextlib

        with contextlib.ExitStack() as ctx:
            const = ctx.enter_context(tc.tile_pool(name="const", bufs=1))
            g_sb = const.tile([128, HP, 2, D], F16)
            nc.sync.dma_start(out=g_sb, in_=qk_g[:])
            vo_wsb = const.tile([128, HP, 2, D], F16)
            nc.sync.dma_start(out=vo_wsb, in_=vo_w[:])
            eps_c = const.tile([128, 1], F32)
            nc.vector.memset(eps_c, 1e-5)

            # persistent activations
            act = ctx.enter_context(tc.tile_pool(name="act", bufs=1))
            ip = act.tile([128, 8, HID], F16, tag="ip")       # token-major, scaled
            tp = act.tile([128, 8, HID], F16, tag="tp")
            ipT = act.tile([128, 8, NTOK], F16, tag="ipT")   # hid-major
            tpT = act.tile([128, 8, NTOK], F16, tag="tpT")

            dqs = [nc.sync, nc.scalar]

            # ---------------- Phase A: projections ----------------
            with contextlib.ExitStack() as pctx:
                xp = pctx.enter_context(tc.tile_pool(name="xT", bufs=1))
                x_i = xp.tile([128, di_k, NTOK], F16, tag="xTi")
                x_t = xp.tile([128, dt_k, NTOK], F16, tag="xTt")
                wp = pctx.enter_context(tc.tile_pool(name="wproj", bufs=1))
                w_i = wp.tile([128, di_k, HID], F16, tag="wi")
                w_t = wp.tile([128, dt_k, HID], F16, tag="wt")
                # text first (smaller; its matmuls can start soonest)
                for k in range(dt_k):
                    nc.sync.dma_start(out=w_t[:, k, :], in_=w_txtT[k * 128:(k + 1) * 128, :])
                    nc.scalar.dma_start(out=x_t[:, k, :], in_=xT_txt[k * 128:(k + 1) * 128, :])
                for k in range(di_k):
                    nc.sync.dma_start(out=w_i[:, k, :], in_=w_imgT[k * 128:(k + 1) * 128, :])
                    nc.scalar.dma_start(out=x_i[:, k, :], in_=xT_img[k * 128:(k + 1) * 128, :])

                ps_p = pctx.enter_context(tc.tile_pool(name="ps_p", bufs=3, space="PSUM"))
                stat = pctx.enter_context(tc.tile_pool(name="pstat", bufs=4))
                for (xsb, kk, w_sb, dst, dstT) in (
                    (x_t, dt_k, w_t, tp, tpT),
                    (x_i, di_k, w_i, ip, ipT),
                ):
                    for t in range(8):
                        pp = ps_p.tile([128, HID], F32, tag="proj")
                        for half in range(2):
                            for k in range(kk):
                                nc.tensor.matmul(
                                    pp[:, half * 512:(half + 1) * 512],
                                    lhsT=xsb[:, k, t * 128:(t + 1) * 128],
                                    rhs=w_sb[:, k, half * 512:(half + 1) * 512],
                                    start=(k == 0),
                                    stop=(k == kk - 1),
                                )
                        # LN stats over 1024 (two 512 subgroups)
                        bstats = stat.tile([128, 2, 6], F32, tag="bst")
                        nc.vector.bn_stats(out=bstats[:, 0, :], in_=pp[:, 0:512])
                        nc.vector.bn_stats(out=bstats[:, 1, :], in_=pp[:, 512:1024])
                        mv = stat.tile([128, 2], F32, tag="mv")
                        nc.vector.bn_aggr(out=mv, in_=bstats)
                        # sv = 1/(256*sqrt(var)) = exp(-0.5*ln(65536*var)) on ACT
                        lv = stat.tile([128, 1], F32, tag="lv")
                        nc.scalar.activation(out=lv, in_=mv[:, 1:2], func=AF.Ln, scale=65536.0)
                        sv = stat.tile([128, 1], F32, tag="sv")
                        nc.scalar.activation(out=sv, in_=lv, func=AF.Exp, scale=-0.5)
                        # nb = -mu*sv (tiny DVE op)
                        nb = stat.tile([128, 1], F32, tag="nb")
                        nc.vector.scalar_tensor_tensor(
                            out=nb, in0=mv[:, 0:1], scalar=-1.0, in1=sv,
                            op0=ALU.mult, op1=ALU.mult,
                        )
                        # ip = x*sv + nb on ScalarE (psum -> sbuf fp16).
                        # Identity (not Copy) so scale/bias may be APs; its
                        # table approximation costs ~1 ULP.
                        nc.scalar.activation(
                            out=dst[:, t, :], in_=pp, func=AF.Identity, scale=sv, bias=nb,
                        )
                        # stream the hid-major transpose chunks for tile t
                        for j in range(8):
                            dqs[(t * 8 + j) % 2].dma_start(
                                out=dstT[:, j, t * 128:(t + 1) * 128],
                                in_=dst[:, t, j * 128:(j + 1) * 128],
                                transpose=True,
                            )

            # ---------------- Phase B: attention + chunked final LN ----------------
            with contextlib.ExitStack() as actx:
                ps_v = actx.enter_context(tc.tile_pool(name="ps_v", bufs=2, space="PSUM"))
                ps_sc = actx.enter_context(tc.tile_pool(name="ps_sc", bufs=2, space="PSUM"))
                ps_o = actx.enter_context(tc.tile_pool(name="ps_o", bufs=2, space="PSUM"))
                sb_u = actx.enter_context(tc.tile_pool(name="sb_u", bufs=2))
                sb_vo = actx.enter_context(tc.tile_pool(name="sb_vo", bufs=2))
                sb_p = actx.enter_context(tc.tile_pool(name="sb_p", bufs=2))
                sb_sm = actx.enter_context(tc.tile_pool(name="sb_sm", bufs=4))
                combp = actx.enter_context(tc.tile_pool(name="combp", bufs=2))
                sqp = actx.enter_context(tc.tile_pool(name="sqp", bufs=2))
                statf = actx.enter_context(tc.tile_pool(name="statf", bufs=4))
                outp = actx.enter_context(tc.tile_pool(name="outp", bufs=2))

                comb = {}
                combacc = {}

                def attention(b, hp_i):
                    u_sb = {}
                    vo_sb = {}
                    pT = {}
                    for dirn in range(2):
                        qT_src = ipT if dirn == 0 else tpT
                        kT_src = tpT if dirn == 0 else ipT
                        ups = ps_v.tile([128, 512], F32, tag="v", name="ups")
                        for h01 in range(2):
                            sl = slice(h01 * 64, (h01 + 1) * 64)
                            nc.tensor.matmul(
                                ups[sl, :],
                                lhsT=g_sb[sl, hp_i, dirn, :],
                                rhs=kT_src[sl, hp_i, b * 512:(b + 1) * 512],
                                start=True, stop=True,
                            )
                        u = sb_u.tile([128, 512], F16, tag="u", name="u")
                        nc.vector.tensor_copy(out=u, in_=ups)
                        u_sb[dirn] = u

                        vps = {}
                        for h01 in range(2):
                            vps[h01] = ps_v.tile([128, 4, D], F32, tag="v", name=f"vps{h01}")
                        for skt in range(4):
                            for h01 in range(2):
                                sl = slice(h01 * 64, (h01 + 1) * 64)
                                nc.tensor.matmul(
                                    vps[h01][:, skt, :],
                                    lhsT=kT_src[sl, hp_i, b * 512 + skt * 128: b * 512 + (skt + 1) * 128],
                                    rhs=vo_wsb[sl, hp_i, dirn, :],
                                    start=True, stop=True,
                                )
                        for h01 in range(2):
                            vos = sb_vo.tile([128, 4, D + 1], F16, tag=f"vo{dirn}{h01}", name=f"vos{h01}")
                            nc.vector.tensor_copy(out=vos[:, :, 0:D], in_=vps[h01])
                            nc.vector.memset(vos[:, :, D:D + 1], 1.0)
                            vo_sb[(dirn, h01)] = vos

                        for h01 in range(2):
                            pT[(dirn, h01)] = sb_p.tile(
                                [128, 4, 512], F16, tag=f"pT{dirn}{h01}", name=f"pt{h01}"
                            )
                        for sp in range(2):
                            scp = {}
                            for h01 in range(2):
                                scp[h01] = ps_sc.tile(
                                    [128, 2, 512], F32, tag="sc", name=f"scp{h01}"
                                )
                            for skh in range(2):
                                skt = sp * 2 + skh
                                for h01 in range(2):
                                    sl = slice(h01 * 64, (h01 + 1) * 64)
                                    nc.tensor.matmul(
                                        scp[h01][:, skh, :],
                                        lhsT=u_sb[dirn][sl, skt * 128:(skt + 1) * 128],
                                        rhs=qT_src[sl, hp_i, b * 512:(b + 1) * 512],
                                        start=True, stop=True,
                                    )
                            for h01 in range(2):
                                nc.scalar.activation(
                                    out=pT[(dirn, h01)][:, sp * 2:(sp + 1) * 2, :],
                                    in_=scp[h01], func=AF.Exp,
                                )

                    for h01 in range(2):
                        h = hp_i * 2 + h01
                        for sqt in range(4):
                            ops = ps_o.tile([128, 2, D + 1], F32, tag="o")
                            for dirn in range(2):
                                for skt in range(4):
                                    nc.tensor.matmul(
                                        ops[:, dirn, :],
                                        lhsT=pT[(dirn, h01)][:, skt, sqt * 128:(sqt + 1) * 128],
                                        rhs=vo_sb[(dirn, h01)][:, skt, :],
                                        start=(skt == 0), stop=(skt == 3),
                                    )
                            rc = sb_sm.tile([128, 2, 1], F32, tag="rc")
                            nc.vector.reciprocal(out=rc, in_=ops[:, :, D:D + 1])
                            for dirn in range(2):
                                src_tm = ip if dirn == 0 else tp
                                nc.vector.scalar_tensor_tensor(
                                    out=comb[(b, sqt)][:, h, dirn * 64:(dirn + 1) * 64],
                                    in0=ops[:, dirn, 0:D],
                                    scalar=rc[:, dirn, 0:1],
                                    in1=src_tm[:, b * 4 + sqt, h * 64:(h + 1) * 64],
                                    op0=ALU.mult, op1=ALU.add,
                                    accum_out=combacc[(b, sqt)][:, h * 2 + dirn: h * 2 + dirn + 1],
                                )

                def finalize_chunk(b, chunk):
                    # chunk = (sqt, head-half): LN over 2D=128 for 8 heads
                    sqt, hh = chunk >> 1, chunk & 1
                    h0 = hh * 8
                    cs = comb[(b, sqt)][:, h0:h0 + 8, :]            # [128, 8, 128]
                    sums = statf.tile([128, 8], F32, tag="sum")
                    nc.vector.tensor_reduce(
                        out=sums,
                        in_=combacc[(b, sqt)][:, h0 * 2:(h0 + 8) * 2].rearrange(
                            "p (h t) -> p h t", t=2),
                        axis=mybir.AxisListType.X, op=ALU.add,
                    )
                    sq = sqp.tile([128, 8, 2 * D], F32, tag="sqot")
                    sq_eng = nc.gpsimd if USE_GPSIMD else nc.vector
                    sq_eng.tensor_tensor(out=sq, in0=cs, in1=cs, op=ALU.mult)
                    sumsq = statf.tile([128, 8], F32, tag="ssq")
                    nc.vector.tensor_reduce(out=sumsq, in_=sq, axis=mybir.AxisListType.X, op=ALU.add)
                    mean = statf.tile([128, 8], F32, tag="mean")
                    nc.vector.tensor_scalar_mul(mean, sums, 1.0 / 128.0)
                    m2 = statf.tile([128, 8], F32, tag="m2")
                    nc.vector.tensor_mul(m2, mean, mean)
                    var = statf.tile([128, 8], F32, tag="var")
                    nc.vector.scalar_tensor_tensor(
                        out=var, in0=sumsq, scalar=1.0 / 128.0, in1=m2,
                        op0=ALU.mult, op1=ALU.subtract,
                    )
                    # rstd = 1/sqrt(var+eps) = exp(-0.5*ln(var+1e-5)) on ACT
                    lv = statf.tile([128, 8], F32, tag="flv")
                    nc.scalar.activation(out=lv, in_=var, func=AF.Ln, bias=eps_c)
                    rstd = statf.tile([128, 8], F32, tag="frs")
                    nc.scalar.activation(out=rstd, in_=lv, func=AF.Exp, scale=-0.5)
                    stage = outp.tile([128, 8, 2 * D], F32, tag="stage")
                    n_eng = nc.gpsimd if USE_GPSIMD else nc.vector
                    n_eng.tensor_tensor(
                        out=stage, in0=cs,
                        in1=mean.to_broadcast([128, 8, 2 * D]), op=ALU.subtract,
                    )
                    n_eng.tensor_tensor(
                        out=stage, in0=stage,
                        in1=rstd.to_broadcast([128, 8, 2 * D]), op=ALU.mult,
                    )
                    dqs[chunk % 2].dma_start(
                        out=out[b, sqt * 128:(sqt + 1) * 128, h0:h0 + 8, :],
                        in_=stage,
                    )

                for b in range(B_LOC):
                    for sqt in range(4):
                        comb[(b, sqt)] = combp.tile(
                            [128, H, 2 * D], F32, tag=f"comb{sqt}", name=f"comb_{b}_{sqt}"
                        )
                        combacc[(b, sqt)] = combp.tile(
                            [128, H * 2], F32, tag=f"cacc{sqt}", name=f"cacc_{b}_{sqt}"
                        )
                    for hp_i in range(HP):
                        attention(b, hp_i)
                        if b == 1:
                            finalize_chunk(0, hp_i)
                for chunk in range(8):
                    finalize_chunk(1, chunk)

    nc.compile()
    return nc


_PROGRAM_CACHE: dict = {}


def _get_program(di_k: int, dt_k: int):
    key = (di_k, dt_k)
    if key not in _PROGRAM_CACHE:
        _PROGRAM_CACHE[key] = _build_program(di_k, dt_k)
    return _PROGRAM_CACHE[key]


def kernel(
    image_features, text_features,
    img_w, img_b, img_ln_g, img_ln_b,
    txt_w, txt_b, txt_ln_g, txt_ln_b,
    i2t_in_w, i2t_in_b, i2t_out_w, i2t_out_b,
    t2i_in_w, t2i_in_b, t2i_out_w, t2i_out_b,
    hn_g, hn_b,
) -> np.ndarray:
    f32 = np.float32
    f16 = np.float16
    image_features = np.asarray(image_features, f32)
    text_features = np.asarray(text_features, f32)

    # --- host-side parameter folding ---
    # The device program implements the default affine paths; non-default
    # LN affines / attention biases are not exercised by this module's
    # parameterization (they are identically zero / one).
    for name, arr, want in (
        ("img_b", img_b, 0.0), ("txt_b", txt_b, 0.0),
        ("img_ln_b", img_ln_b, 0.0), ("txt_ln_b", txt_ln_b, 0.0),
        ("i2t_in_b", i2t_in_b, 0.0), ("i2t_out_b", i2t_out_b, 0.0),
        ("t2i_in_b", t2i_in_b, 0.0), ("t2i_out_b", t2i_out_b, 0.0),
        ("hn_b", hn_b, 0.0),
    ):
        if np.any(np.asarray(arr) != want):
            if name in ("img_b", "txt_b"):
                continue  # handled via input padding below
            raise NotImplementedError(f"nonzero {name} not supported")
    for name, arr in (("img_ln_g", img_ln_g), ("txt_ln_g", txt_ln_g), ("hn_g", hn_g)):
        if np.any(np.asarray(arr) != 1.0):
            raise NotImplementedError(f"non-unit {name} not supported")

    def prep_x_w(x, w, bvec):
        # fold projection bias via an extra zero-padded 128-block with a
        # ones column at position 0 of the block
        d = x.shape[2]
        xf = np.ascontiguousarray(x.reshape(B, S, d))
        wT = np.ascontiguousarray(w.T.astype(f32))  # [d, HID]
        if np.any(np.asarray(bvec) != 0.0):
            xf = np.concatenate(
                [xf, np.zeros((B, S, 128), f32)], axis=2)
            xf[:, :, d] = 1.0
            wT = np.concatenate([wT, np.zeros((128, HID), f32)], axis=0)
            wT[d, :] = np.asarray(bvec, f32)
        return np.ascontiguousarray(xf), np.ascontiguousarray(wT)

    xi, wiT = prep_x_w(image_features, img_w, img_b)
    xt, wtT = prep_x_w(text_features, txt_w, txt_b)
    wiT = wiT.astype(f16)
    wtT = wtT.astype(f16)
    di_k = xi.shape[2] // 128
    dt_k = xt.shape[2] // 128

    # per-head folded attention weights
    qk_g = np.zeros((128, HP, 2, D), f32)
    vo_w = np.zeros((128, HP, 2, D), f32)
    for h in range(H):
        hp_i, h01 = h // 2, h % 2
        sl = slice(h01 * 64, (h01 + 1) * 64)
        for dirn, (in_w, out_w) in enumerate(
            ((i2t_in_w, i2t_out_w), (t2i_in_w, t2i_out_w))
        ):
            Wq = np.asarray(in_w[h][:D], f32)       # [e, d_q]
            Wk = np.asarray(in_w[h][D:2 * D], f32)  # [e, d_k]
            Wv = np.asarray(in_w[h][2 * D:], f32)   # [e, d_v]
            Ow = np.asarray(out_w[h], f32)          # [f, e]
            qk_g[sl, hp_i, dirn, :] = 0.125 * (Wk.T @ Wq)   # [d_k, d_q]
            vo_w[sl, hp_i, dirn, :] = (Ow @ Wv).T            # [d, f]
    qk_g = qk_g.astype(f16)
    vo_w = vo_w.astype(f16)

    nc = _get_program(di_k, dt_k)

    in_maps = []
    for c in range(NCORES):
        bs = slice(c * B_LOC, (c + 1) * B_LOC)
        # host-side transpose to feature-major fp16
        xTi = np.ascontiguousarray(xi[bs].reshape(NTOK, -1).T.astype(f16))
        xTt = np.ascontiguousarray(xt[bs].reshape(NTOK, -1).T.astype(f16))
        in_maps.append({
            "xT_img": xTi,
            "xT_txt": xTt,
            "w_imgT": wiT,
            "w_txtT": wtT,
            "qk_g": qk_g,
            "vo_w": vo_w,
        })

    res = run_bass_kernel_spmd(nc, in_maps, core_ids=list(range(NCORES)))
    global LAST_EXEC_NS, LAST_RESULT
    LAST_RESULT = res
    LAST_EXEC_NS = getattr(res, "exec_time_ns", None)
    # device output is token-major [B_LOC, S, H, 2D]
    out = np.concatenate([r["out"] for r in res.results], axis=0)
    out = np.ascontiguousarray(out.transpose(0, 2, 1, 3))
    return out.astype(f32)


LAST_EXEC_NS = None
LAST_RESULT = None
